# revision 25
# baseline (speedup 1.0000x reference)
"""Multi-head attention (B=2, N=2048, dim=1024, heads=16, dim_head=64) on
8 TRN2 NeuronCores.

Sharding: data-parallel over batch (2) x tensor-parallel over heads (4 per
core).  Core c handles batch b = c//4 and heads [4g, 4g+4), g = c%4.  Each
core computes its 4 heads' attention plus the partial out-projection
(O_heads @ w_out[head rows]); the host sums the 4 partials per batch and
adds the bias.

Per-core device algorithm (bf16 matmul inputs, fp32 PSUM accumulate; exp
without max-subtraction — scores are ~N(0,1) so exp never overflows):
  xT  [1024, 2048] = x[b].T               (transposed on host, free)
  Qt/Kt [128(i of head pair), 2048(n)] = w_slice.T @ xT    (W stationary)
  V   [128(m), 16(mt), 4(h), 65]  natural layout, col 64 = ones so the
       attention-value matmul also produces the softmax denominator.
  per head pair (A,B packed in PE rows 0-63 / 64-127 via tile_position),
  per 512-wide query chunk, per key tile mt:
       St[h] = Kt_h[64, 128].T @ Qt_h[64, 512] -> PSUM [128, 1024] (h0|h1
               in two adjacent banks of one wide tile)
       Pt = exp(St * 1/8)                      -> one [128, 1024] ScalarE
               activation covering both heads (halves ACT instruction count)
       Ot' += V'[128, 65].T @ Pt[:, 512h:+512] -> PSUM [65, 512] accum
  normalize: o = Ot'[0:64] * (1 / Ot'[64]) (DVE recip + GPSIMD bcast + DVE)
  proj: out[nt*128:+128, jc*512:+512] = sum_p o_sb[:,p,nt].T @ wo[:,p,jc]

Schedule: input DMAs ordered wqkv, xT chunk 0..3, wo; QKV projection units
emitted per-chunk as the xT chunks land (dummy matmuls bridge the initial
DMA-only window so the HAM clock gate never throttles); attention pair 0
starts as soon as Kt pair 0 is complete (~20us), with the remaining QKV
units as fillers; attention pair 1 carries the out-projection as filler.
"""
import numpy as np

import concourse.bass as bass
import concourse.mybir as mybir
import concourse.tile as tile
from concourse import bacc
from concourse.bass_utils import run_bass_kernel_spmd

# Problem constants (hardcoded per contract).
B = 2
N = 2048
DIM = 1024
HEADS = 16
DH = 64
INNER = HEADS * DH
SCALE = DH ** -0.5

N_CORES = 8
HEADS_PER_CORE = 4
PAIRS = 2          # head pairs per core
NT = N // 128      # 16 key/query tiles
DT = DIM // 128    # 8 contraction tiles
CH = N // 512      # 4 query chunks
F32 = mybir.dt.float32
F32R = mybir.dt.float32r
BF16 = mybir.dt.bfloat16

_CACHED_NC = None


def _emit_kernel(tc, xt_d, wqkv_d, wo_d, out_d):
    nc = tc.nc

    from contextlib import ExitStack

    ctx = ExitStack()
    per = ctx.enter_context(tc.tile_pool(name="persist", bufs=1))
    psum = ctx.enter_context(tc.tile_pool(name="psum", bufs=1, space="PSUM"))
    work = ctx.enter_context(tc.tile_pool(name="work", bufs=1))

    # Persistent SBUF tensors.
    xt_sb = per.tile([128, DT, N], BF16, tag="xt")
    wqkv_sb = per.tile([128, DT, 768], BF16, tag="wqkv")
    wo_sb = per.tile([128, PAIRS, DIM], BF16, tag="wo")
    qt_sb = per.tile([128, PAIRS, N], BF16, tag="qt")
    kt_sb = per.tile([128, PAIRS, N], BF16, tag="kt")
    v_sb = per.tile([128, NT, HEADS_PER_CORE, DH + 1], BF16, tag="v")
    o_sb = per.tile([128, PAIRS, N], BF16, tag="o")

    # Input DMAs in consumption order: wqkv (needed by every projection
    # unit), xT chunk by chunk, wo last (first used ~100us in).  One wide
    # DMA per tensor/chunk — a single instruction's packets already spread
    # over all DMA engines, and fewer issues (6 vs 24) gets every transfer
    # in flight within ~4us of kernel start.
    # Per-dt DMAs (a wide rearranged DMA decodes wrong on the hardware
    # DGE), spread across two issuing engines so all input transfers are
    # in flight within ~8us instead of ~15us of serial issue on Sync.
    for dt in range(DT):
        nc.sync.dma_start(wqkv_sb[:, dt, :], wqkv_d[128 * dt:128 * (dt + 1), :])
    for c in range(CH):
        eng = nc.sync if c % 2 == 0 else nc.gpsimd
        for dt in range(DT):
            eng.dma_start(
                xt_sb[:, dt, 512 * c:512 * (c + 1)],
                xt_d[128 * dt:128 * (dt + 1), 512 * c:512 * (c + 1)],
            )
    for p in range(PAIRS):
        nc.gpsimd.dma_start(wo_sb[:, p, :], wo_d[128 * p:128 * (p + 1), :])

    # Ones column of V' (gives the softmax denominator through the AV matmul).
    ones_sb = per.tile([128, NT * HEADS_PER_CORE], F32, tag="ones")
    nc.vector.memset(ones_sb[:], 1.0)
    nc.vector.tensor_copy(
        v_sb[:, :, :, DH:DH + 1],
        ones_sb[:].rearrange("p (a b c) -> p a b c", b=HEADS_PER_CORE, c=1),
    )
    # Touch Exp once so the ACT table DMA (~1.3us + pseudo-load) happens
    # during the startup phase rather than before the first real exp.
    warm = work.tile([1, 1], F32, tag="warm")
    nc.scalar.activation(
        warm[:], ones_sb[0:1, 0:1], mybir.ActivationFunctionType.Exp, scale=1.0
    )

    def emit_dummies(n):
        # Keep the PE busy through DMA-only windows so the HAM clock gate
        # stays at full speed (an idle PE gets throttled to 1.2 GHz for
        # several us).  Output is never read.
        for _ in range(n):
            dmy = psum.tile([64, 64], F32, tag="stp", bufs=2, name="dummy")
            nc.tensor.matmul(
                dmy[:], ones_sb[:, 0:64], ones_sb[:, 0:64], start=True, stop=True
            )

    def emit_qk_chunk(which, p, c):
        """Qt or Kt for head pair p, n-chunk c: [128, 512] of W.T @ xT."""
        src = qt_sb if which == "q" else kt_sb
        col0 = (0 if which == "q" else 256) + 128 * p
        ps = psum.tile([128, 512], F32, tag="qk", bufs=2)
        for dt in range(DT):
            nc.tensor.matmul(
                ps[:],
                wqkv_sb[:, dt, col0:col0 + 128],
                xt_sb[:, dt, 512 * c:512 * (c + 1)],
                start=(dt == 0),
                stop=(dt == DT - 1),
            )
        nc.vector.tensor_copy(src[:, p, 512 * c:512 * (c + 1)], ps[:])

    def emit_v_tile(mt):
        """V natural [128(m), 256(4 heads x 64)] for key tile mt."""
        ps = psum.tile([128, 256], F32, tag="qk", bufs=2)
        for dt in range(DT):
            nc.tensor.matmul(
                ps[:],
                xt_sb[:, dt, 128 * mt:128 * (mt + 1)],
                wqkv_sb[:, dt, 512:768],
                start=(dt == 0),
                stop=(dt == DT - 1),
            )
        nc.vector.tensor_copy(
            v_sb[:, mt, :, 0:DH],
            ps[:].rearrange("p (h d) -> p h d", h=HEADS_PER_CORE),
        )

    ev_tiles = {}

    def emit_proj_unit(nt, jc, evac=None):
        """out[128nt:+128, 512jc:+512] = sum_p o_sb[:,p,nt].T @ wo[:,p,jc].

        Both jc halves of a row tile land in one [128, 1024] bf16 staging
        tile; the output DMA is split in two ops on two issuing engines so
        the tail transfer overlaps itself across DMA rings.
        """
        if nt not in ev_tiles:
            ev_tiles[nt] = work.tile([128, DIM], BF16, tag="ev", bufs=4, name="ev")
        ev = ev_tiles[nt]
        ps = psum.tile([128, 512], F32, tag="qk", bufs=2)
        for p in range(PAIRS):
            nc.tensor.matmul(
                ps[:],
                o_sb[:, p, 128 * nt:128 * (nt + 1)],
                wo_sb[:, p, 512 * jc:512 * (jc + 1)],
                start=(p == 0),
                stop=(p == PAIRS - 1),
            )
        if evac == "scalar":
            nc.scalar.copy(ev[:, 512 * jc:512 * (jc + 1)], ps[:])
        else:
            nc.vector.tensor_copy(ev[:, 512 * jc:512 * (jc + 1)], ps[:])
        if jc == 1:
            r0 = 128 * nt
            nc.sync.dma_start(out_d[r0:r0 + 64, :], ev[0:64, :])
            nc.gpsimd.dma_start(out_d[r0 + 64:r0 + 128, :], ev[64:128, :])
            del ev_tiles[nt]

    def emit_unit(u):
        if u[0] == "v":
            emit_v_tile(u[1])
        elif u[0] == "qk":
            emit_qk_chunk(u[1], u[2], u[3])
        elif u[0] == "proj":
            emit_proj_unit(u[1], u[2])
        elif u[0] == "dummy":
            emit_dummies(u[1])

    def emit_normalize(ot, p, c, engine="vector"):
        """Normalize: o = Ot'[0:64] / Ot'[64].  First evacuate each head's
        Ot' to SBUF (denominator row to a partition-0 tile, numerator via
        one wide copy) — that frees the ot PSUM bank after ~1.1us so the
        next chunk's first AV matmul never waits on the rest of the chain
        (recip/broadcast/mul).  engine="scalar" moves the copies to the
        otherwise-idle ScalarE during the drain.

        (reciprocal_approx_fast misbehaves on hardware when its input AP
        sits at a nonzero base partition, so den gets a partition-0 copy.)
        """
        copy = (nc.scalar.copy if engine == "scalar"
                else lambda o, i: nc.vector.tensor_copy(o, i))
        den, otmp, recip, rbc = ([None, None] for _ in range(4))
        for h in range(2):
            den[h] = work.tile([1, 512], F32, tag="den", bufs=4, name=f"den{h}")
            copy(den[h][:], ot[h][DH:DH + 1, :])
            otmp[h] = work.tile([DH, 512], F32, tag="otmp", bufs=4, name=f"otmp{h}")
            copy(otmp[h][:], ot[h][0:DH, :])
        for h in range(2):
            recip[h] = work.tile([1, 512], F32, tag="recip", bufs=4, name=f"rec{h}")
            nc.vector.reciprocal_approx_fast(recip[h][:], den[h][:])
        for h in range(2):
            rbc[h] = work.tile([64, 512], F32, tag="rbc", bufs=4, name=f"rbc{h}")
            nc.gpsimd.partition_broadcast(rbc[h][:], recip[h][:])
        for h in range(2):
            nc.vector.tensor_mul(
                o_sb[64 * h:64 * (h + 1), p, 512 * c:512 * (c + 1)],
                otmp[h][:],
                rbc[h][:],
            )

    # AV matmuls lag the St/exp of the current key tile by two slots so the
    # in-order PE queue never head-of-line blocks on ScalarE.  The pending
    # list carries across chunk boundaries (software pipelining): a chunk's
    # last two AVs and its normalize are emitted during the next chunk's
    # first two slots, by which time its last exp has certainly finished.
    pending = []

    def flush_one(last_norm_engine="vector"):
        e = pending.pop(0)
        for h in range(2):
            nc.tensor.matmul(
                e["ot"][h][:],
                v_sb[:, e["mt"], 2 * e["p"] + h, :],
                e["pt"][:, 512 * h:512 * (h + 1)],
                start=(e["mt"] == 0),
                stop=(e["mt"] == NT - 1),
            )
        if e["mt"] == NT - 1:
            emit_normalize(e["ot"], e["p"], e["c"], engine=last_norm_engine)

    def emit_att_chunk(p, c, filler=None):
        """Attention for head pair p, query chunk c (cols 512c..512c+512).

        The two heads of a pair sit in PE rows 0-63 / 64-127 (tile_position
        row packing); their St outputs land in the two adjacent PSUM banks
        of one [128, 1024] tile so a single wide ScalarE activation
        exponentiates both.
        """
        ot = [
            psum.tile([DH + 1, 512], F32, tag="ot", bufs=2, name=f"ot{h}")
            for h in range(2)
        ]
        for mt in range(NT):
            stp = psum.tile([128, 1024], F32, tag="stp", bufs=2, name="stp")
            for h in range(2):
                nc.tensor.matmul(
                    stp[:, 512 * h:512 * (h + 1)],
                    kt_sb[64 * h:64 * (h + 1), p, 128 * mt:128 * (mt + 1)],
                    qt_sb[64 * h:64 * (h + 1), p, 512 * c:512 * (c + 1)],
                    start=True,
                    stop=True,
                    tile_position=(64 * h, 0),
                )
            pt = work.tile([128, 1024], BF16, tag="pt", bufs=8, name="pt")
            nc.scalar.activation(
                pt[:], stp[:], mybir.ActivationFunctionType.Exp, scale=SCALE
            )
            pending.append({"ot": ot, "p": p, "c": c, "mt": mt, "pt": pt})
            if len(pending) == 3:
                flush_one()
            if filler is not None:
                filler(c, mt)

    # ---- Emission schedule ----
    # Phase B: dummy matmuls cover the initial DMA window (wqkv + xT chunk
    # 0 land ~8us in), then Kt pair 0 per xT-chunk arrival (the critical
    # path to attention start), Qt(p0, c0), and the first two V tiles.
    # Dummies between the Kt units bridge the xT chunk arrival gaps (the PE
    # would otherwise idle on the DMA semaphore and trip the HAM throttle).
    phase_b = [
        ("dummy", 34),
        ("qk", "k", 0, 0), ("qk", "q", 0, 0), ("v", 0), ("v", 1),
        ("dummy", 20), ("qk", "k", 0, 1),
        ("dummy", 6), ("qk", "k", 0, 2),
        ("dummy", 6), ("qk", "k", 0, 3),
    ]
    for u in phase_b:
        emit_unit(u)

    # Fillers (slot = mt index, NT = after last exp).  Fillers sit at the
    # START of each chunk: the first AV of a chunk waits ~1.2us for the
    # previous chunk's ot evacuation, and St(mt=1) waits for the previous
    # chunk's last exp to free its stp slot — front-loaded fillers absorb
    # both so the PE never idles (an idle PE trips the HAM clock throttle).
    # NOTE: inside emit_att_chunk the AV flush for key tiles (mt-2, mt-1)
    # is emitted BEFORE the slot-mt filler, so V tile j must sit at slot
    # <= j+1 (one earlier to cover the DVE evac latency).
    att0_fill = {
        0: {0: [("v", 2), ("v", 3)], 1: [("v", 4), ("v", 5)],
            2: [("v", 6)], 3: [("v", 7)], 5: [("v", 8), ("v", 9)],
            7: [("v", 10), ("v", 11)], 9: [("v", 12), ("v", 13)],
            11: [("v", 14), ("v", 15)], 13: [("qk", "q", 0, 1)]},
        1: {0: [("qk", "k", 1, 0)], 1: [("qk", "q", 0, 2)],
            8: [("qk", "k", 1, 1)]},
        2: {0: [("qk", "k", 1, 2)], 1: [("qk", "q", 0, 3)],
            8: [("qk", "q", 1, 0)]},
        3: {0: [("qk", "k", 1, 3)], 8: [("qk", "q", 1, 1)]},
    }

    def att0_filler(c, mt):
        for u in att0_fill[c].get(mt, ()):
            emit_unit(u)

    for c in range(CH):
        emit_att_chunk(0, c, filler=att0_filler)

    # Phase D: attention pair 1.  Chunk 0 finishes the last Qt unit;
    # chunks 1-3 carry the out-projection for the query rows of chunk c-1
    # (complete for both pairs by then), front-loaded for the same reason.
    att1_fill = {
        0: {0: [("qk", "q", 1, 2)], 8: [("qk", "q", 1, 3)]},
    }
    # proj fillers start at slot 2: the previous chunk's normalize is only
    # emitted during this chunk's slot-1 flush, and proj reads its o rows.
    for c in range(1, CH):
        units = [("proj", nt, jc)
                 for nt in range(4 * (c - 1), 4 * c) for jc in range(2)]
        slots = [2, 2, 3, 3, 4, 6, 8, 10]
        att1_fill[c] = {}
        for s, u in zip(slots, units):
            att1_fill[c].setdefault(s, []).append(u)

    def att1_filler(c, mt):
        for u in att1_fill.get(c, {}).get(mt, ()):
            emit_unit(u)

    for c in range(CH):
        emit_att_chunk(1, c, filler=att1_filler)
    # Drain the carried AVs of the last chunk; its normalize copies go to
    # ScalarE (idle after the last exp) so the DVE backlog of proj
    # evacuations never delays the final proj units.
    while pending:
        flush_one(last_norm_engine="scalar")

    # Drain: last four row tiles.  Alternate the PSUM evacuations between
    # ScalarE and DVE to halve the tail.
    i = 0
    for nt in range(12, 16):
        for jc in range(2):
            emit_proj_unit(nt, jc, evac="scalar" if i % 2 else None)
            i += 1

    ctx.close()


def _build():
    global _CACHED_NC
    if _CACHED_NC is not None:
        return _CACHED_NC
    nc = bacc.Bacc(
        "TRN2",
        target_bir_lowering=False,
        debug=False,
        enable_asserts=True,
        num_devices=N_CORES,
    )
    xt_d = nc.dram_tensor("xt", [DIM, N], BF16, kind="ExternalInput").ap()
    wqkv_d = nc.dram_tensor("wqkv", [DIM, 768], BF16, kind="ExternalInput").ap()
    wo_d = nc.dram_tensor("wo", [256, DIM], BF16, kind="ExternalInput").ap()
    out_d = nc.dram_tensor("out", [N, DIM], BF16, kind="ExternalOutput").ap()

    with tile.TileContext(nc) as tc:
        _emit_kernel(tc, xt_d, wqkv_d, wo_d, out_d)
    nc.compile()
    _CACHED_NC = nc
    return nc


def _in_maps(x, w_qkv, w_out):
    import ml_dtypes

    bf = ml_dtypes.bfloat16
    maps = []
    for c in range(N_CORES):
        b, g = divmod(c, 4)
        cols = slice(256 * g, 256 * (g + 1))
        wqkv_c = np.ascontiguousarray(
            np.concatenate(
                [
                    w_qkv[:, cols],
                    w_qkv[:, INNER:][:, cols],
                    w_qkv[:, 2 * INNER:][:, cols],
                ],
                axis=1,
            ).astype(bf)
        )
        maps.append(
            {
                "xt": np.ascontiguousarray(x[b].T.astype(bf)),
                "wqkv": wqkv_c,
                "wo": np.ascontiguousarray(w_out[cols, :].astype(bf)),
            }
        )
    return maps


def _run(x, w_qkv, w_out, b_out, trace=False):
    nc = _build()
    res = run_bass_kernel_spmd(
        nc, _in_maps(x, w_qkv, w_out), list(range(N_CORES)), trace=trace
    )
    partials = np.stack(
        [np.asarray(res.results[c]["out"], dtype=np.float32)
         for c in range(N_CORES)]
    )
    out = np.empty((B, N, DIM), dtype=np.float32)
    for b in range(B):
        out[b] = partials[4 * b:4 * b + 4].sum(axis=0) + b_out
    return out, res


def kernel(x, w_qkv, w_out, b_out):
    out, _ = _run(
        np.asarray(x, dtype=np.float32),
        np.asarray(w_qkv, dtype=np.float32),
        np.asarray(w_out, dtype=np.float32),
        np.asarray(b_out, dtype=np.float32),
    )
    return out


# revision 30
# speedup vs baseline: 1.0093x; 1.0093x over previous
"""Multi-head attention (B=2, N=2048, dim=1024, heads=16, dim_head=64) on
8 TRN2 NeuronCores.

Sharding: data-parallel over batch (2) x tensor-parallel over heads (4 per
core).  Core c handles batch b = c//4 and heads [4g, 4g+4), g = c%4.  Each
core computes its 4 heads' attention plus the partial out-projection
(O_heads @ w_out[head rows]); the host sums the 4 partials per batch and
adds the bias.

Per-core device algorithm (bf16 matmul inputs, fp32 PSUM accumulate; exp
without max-subtraction — scores are ~N(0,1) so exp never overflows):
  xT  [1024, 2048] = x[b].T               (transposed on host, free)
  Qt/Kt [128(i of head pair), 2048(n)] = w_slice.T @ xT    (W stationary)
  V   [128(m), 16(mt), 4(h), 65]  natural layout, col 64 = ones so the
       attention-value matmul also produces the softmax denominator.
  per head pair (A,B packed in PE rows 0-63 / 64-127 via tile_position),
  per 512-wide query chunk, per key tile mt:
       St[h] = Kt_h[64, 128].T @ Qt_h[64, 512] -> PSUM [128, 1024] (h0|h1
               in two adjacent banks of one wide tile)
       Pt = exp(St * 1/8)                      -> one [128, 1024] ScalarE
               activation covering both heads (halves ACT instruction count)
       Ot' += V'[128, 65].T @ Pt[:, 512h:+512] -> PSUM [65, 512] accum
  normalize: o = Ot'[0:64] * (1 / Ot'[64]) (DVE recip + GPSIMD bcast + DVE)
  proj: out[nt*128:+128, jc*512:+512] = sum_p o_sb[:,p,nt].T @ wo[:,p,jc]

Schedule: input DMAs ordered wqkv, xT chunk 0..3, wo; QKV projection units
emitted per-chunk as the xT chunks land (dummy matmuls bridge the initial
DMA-only window so the HAM clock gate never throttles); attention pair 0
starts as soon as Kt pair 0 is complete (~20us), with the remaining QKV
units as fillers; attention pair 1 carries the out-projection as filler.
"""
import numpy as np

import concourse.bass as bass
import concourse.mybir as mybir
import concourse.tile as tile
from concourse import bacc
from concourse.bass_utils import run_bass_kernel_spmd

# Problem constants (hardcoded per contract).
B = 2
N = 2048
DIM = 1024
HEADS = 16
DH = 64
INNER = HEADS * DH
SCALE = DH ** -0.5

N_CORES = 8
HEADS_PER_CORE = 4
PAIRS = 2          # head pairs per core
NT = N // 128      # 16 key/query tiles
DT = DIM // 128    # 8 contraction tiles
CH = N // 512      # 4 query chunks
F32 = mybir.dt.float32
F32R = mybir.dt.float32r
BF16 = mybir.dt.bfloat16

_CACHED_NC = None


def _emit_kernel(tc, xt_d, wqkv_d, wo_d, out_d):
    nc = tc.nc

    from contextlib import ExitStack

    ctx = ExitStack()
    per = ctx.enter_context(tc.tile_pool(name="persist", bufs=1))
    psum = ctx.enter_context(tc.tile_pool(name="psum", bufs=1, space="PSUM"))
    work = ctx.enter_context(tc.tile_pool(name="work", bufs=1))

    # Persistent SBUF tensors.
    xt_sb = per.tile([128, DT, N], BF16, tag="xt")
    wqkv_sb = per.tile([128, DT, 768], BF16, tag="wqkv")
    wo_sb = per.tile([128, PAIRS, DIM], BF16, tag="wo")
    qt_sb = per.tile([128, PAIRS, N], BF16, tag="qt")
    kt_sb = per.tile([128, PAIRS, N], BF16, tag="kt")
    v_sb = per.tile([128, NT, HEADS_PER_CORE, DH + 1], BF16, tag="v")
    o_sb = per.tile([128, PAIRS, N], BF16, tag="o")

    # Input DMAs in consumption order: wqkv (needed by every projection
    # unit), xT chunk by chunk, wo last (first used ~100us in).  One wide
    # DMA per tensor/chunk — a single instruction's packets already spread
    # over all DMA engines, and fewer issues (6 vs 24) gets every transfer
    # in flight within ~4us of kernel start.
    # Per-dt DMAs (a wide rearranged DMA decodes wrong on the hardware
    # DGE), split across the two HWDGE issuing engines (Sync + ScalarE,
    # idle until the first exp) so all input transfers are in flight
    # within ~8us instead of ~15us of serial issue on Sync.
    for dt in range(DT):
        nc.sync.dma_start(wqkv_sb[:, dt, :], wqkv_d[128 * dt:128 * (dt + 1), :])
    for c in range(CH):
        eng = nc.sync if c % 2 == 0 else nc.scalar
        for dt in range(DT):
            eng.dma_start(
                xt_sb[:, dt, 512 * c:512 * (c + 1)],
                xt_d[128 * dt:128 * (dt + 1), 512 * c:512 * (c + 1)],
            )
    for p in range(PAIRS):
        nc.scalar.dma_start(wo_sb[:, p, :], wo_d[128 * p:128 * (p + 1), :])

    # Ones column of V' (gives the softmax denominator through the AV matmul).
    ones_sb = per.tile([128, NT * HEADS_PER_CORE], F32, tag="ones")
    nc.vector.memset(ones_sb[:], 1.0)
    nc.vector.tensor_copy(
        v_sb[:, :, :, DH:DH + 1],
        ones_sb[:].rearrange("p (a b c) -> p a b c", b=HEADS_PER_CORE, c=1),
    )
    # Touch Exp once so the ACT table DMA (~1.3us + pseudo-load) happens
    # during the startup phase rather than before the first real exp.
    warm = work.tile([1, 1], F32, tag="warm")
    nc.scalar.activation(
        warm[:], ones_sb[0:1, 0:1], mybir.ActivationFunctionType.Exp, scale=1.0
    )

    def emit_dummies(n):
        # Keep the PE busy through DMA-only windows so the HAM clock gate
        # stays at full speed (an idle PE gets throttled to 1.2 GHz for
        # several us).  Output is never read.
        for _ in range(n):
            dmy = psum.tile([64, 64], F32, tag="stp", bufs=2, name="dummy")
            nc.tensor.matmul(
                dmy[:], ones_sb[:, 0:64], ones_sb[:, 0:64], start=True, stop=True
            )

    def emit_qk_chunk(which, p, c):
        """Qt or Kt for head pair p, n-chunk c: [128, 512] of W.T @ xT."""
        src = qt_sb if which == "q" else kt_sb
        col0 = (0 if which == "q" else 256) + 128 * p
        ps = psum.tile([128, 512], F32, tag="qk", bufs=2)
        for dt in range(DT):
            nc.tensor.matmul(
                ps[:],
                wqkv_sb[:, dt, col0:col0 + 128],
                xt_sb[:, dt, 512 * c:512 * (c + 1)],
                start=(dt == 0),
                stop=(dt == DT - 1),
            )
        nc.vector.tensor_copy(src[:, p, 512 * c:512 * (c + 1)], ps[:])

    def emit_v_tile(mt):
        """V natural [128(m), 256(4 heads x 64)] for key tile mt."""
        ps = psum.tile([128, 256], F32, tag="qk", bufs=2)
        for dt in range(DT):
            nc.tensor.matmul(
                ps[:],
                xt_sb[:, dt, 128 * mt:128 * (mt + 1)],
                wqkv_sb[:, dt, 512:768],
                start=(dt == 0),
                stop=(dt == DT - 1),
            )
        nc.vector.tensor_copy(
            v_sb[:, mt, :, 0:DH],
            ps[:].rearrange("p (h d) -> p h d", h=HEADS_PER_CORE),
        )

    ev_tiles = {}

    def emit_proj_unit(nt, jc, evac=None):
        """out[128nt:+128, 512jc:+512] = sum_p o_sb[:,p,nt].T @ wo[:,p,jc].

        Both jc halves of a row tile land in one [128, 1024] bf16 staging
        tile; the output DMA is split in two ops on two issuing engines so
        the tail transfer overlaps itself across DMA rings.
        """
        if nt not in ev_tiles:
            ev_tiles[nt] = work.tile([128, DIM], BF16, tag="ev", bufs=4, name="ev")
        ev = ev_tiles[nt]
        ps = psum.tile([128, 512], F32, tag="qk", bufs=2)
        for p in range(PAIRS):
            nc.tensor.matmul(
                ps[:],
                o_sb[:, p, 128 * nt:128 * (nt + 1)],
                wo_sb[:, p, 512 * jc:512 * (jc + 1)],
                start=(p == 0),
                stop=(p == PAIRS - 1),
            )
        if evac == "scalar":
            nc.scalar.copy(ev[:, 512 * jc:512 * (jc + 1)], ps[:])
        else:
            nc.vector.tensor_copy(ev[:, 512 * jc:512 * (jc + 1)], ps[:])
        if jc == 1:
            r0 = 128 * nt
            if evac is None:
                nc.sync.dma_start(out_d[r0:r0 + 128, :], ev[:])
            else:
                # Drain: split by rows across both HWDGE engines so the
                # final transfers overlap across DMA rings.
                nc.sync.dma_start(out_d[r0:r0 + 64, :], ev[0:64, :])
                nc.scalar.dma_start(out_d[r0 + 64:r0 + 128, :], ev[64:128, :])
            del ev_tiles[nt]

    def emit_unit(u):
        if u[0] == "v":
            emit_v_tile(u[1])
        elif u[0] == "qk":
            emit_qk_chunk(u[1], u[2], u[3])
        elif u[0] == "proj":
            emit_proj_unit(u[1], u[2])
        elif u[0] == "dummy":
            emit_dummies(u[1])

    def emit_normalize(ot, p, c, engine="vector"):
        """Normalize: o = Ot'[0:64] / Ot'[64].  First evacuate each head's
        Ot' to SBUF (denominator row to a partition-0 tile, numerator via
        one wide copy) — that frees the ot PSUM bank after ~1.1us so the
        next chunk's first AV matmul never waits on the rest of the chain
        (recip/broadcast/mul).  engine="scalar" moves the copies to the
        otherwise-idle ScalarE during the drain.

        (reciprocal_approx_fast misbehaves on hardware when its input AP
        sits at a nonzero base partition, so den gets a partition-0 copy.)
        """
        copy = (nc.scalar.copy if engine == "scalar"
                else lambda o, i: nc.vector.tensor_copy(o, i))
        den, otmp, recip, rbc = ([None, None] for _ in range(4))
        for h in range(2):
            den[h] = work.tile([1, 512], F32, tag="den", bufs=4, name=f"den{h}")
            copy(den[h][:], ot[h][DH:DH + 1, :])
            otmp[h] = work.tile([DH, 512], F32, tag="otmp", bufs=4, name=f"otmp{h}")
            copy(otmp[h][:], ot[h][0:DH, :])
        for h in range(2):
            recip[h] = work.tile([1, 512], F32, tag="recip", bufs=4, name=f"rec{h}")
            nc.vector.reciprocal_approx_fast(recip[h][:], den[h][:])
        for h in range(2):
            rbc[h] = work.tile([64, 512], F32, tag="rbc", bufs=4, name=f"rbc{h}")
            nc.gpsimd.partition_broadcast(rbc[h][:], recip[h][:])
        for h in range(2):
            nc.vector.tensor_mul(
                o_sb[64 * h:64 * (h + 1), p, 512 * c:512 * (c + 1)],
                otmp[h][:],
                rbc[h][:],
            )

    # AV matmuls lag the St/exp of the current key tile by two slots so the
    # in-order PE queue never head-of-line blocks on ScalarE.  The pending
    # list carries across chunk boundaries (software pipelining): a chunk's
    # last two AVs and its normalize are emitted during the next chunk's
    # first two slots, by which time its last exp has certainly finished.
    pending = []

    def flush_one(last_norm_engine="vector"):
        e = pending.pop(0)
        for h in range(2):
            nc.tensor.matmul(
                e["ot"][h][:],
                v_sb[:, e["mt"], 2 * e["p"] + h, :],
                e["pt"][:, 512 * h:512 * (h + 1)],
                start=(e["mt"] == 0),
                stop=(e["mt"] == NT - 1),
            )
        if e["mt"] == NT - 1:
            emit_normalize(e["ot"], e["p"], e["c"], engine=last_norm_engine)

    def emit_att_chunk(p, c, filler=None):
        """Attention for head pair p, query chunk c (cols 512c..512c+512).

        The two heads of a pair sit in PE rows 0-63 / 64-127 (tile_position
        row packing); their St outputs land in the two adjacent PSUM banks
        of one [128, 1024] tile so a single wide ScalarE activation
        exponentiates both.
        """
        ot = [
            psum.tile([DH + 1, 512], F32, tag="ot", bufs=2, name=f"ot{h}")
            for h in range(2)
        ]
        for mt in range(NT):
            stp = psum.tile([128, 1024], F32, tag="stp", bufs=2, name="stp")
            for h in range(2):
                nc.tensor.matmul(
                    stp[:, 512 * h:512 * (h + 1)],
                    kt_sb[64 * h:64 * (h + 1), p, 128 * mt:128 * (mt + 1)],
                    qt_sb[64 * h:64 * (h + 1), p, 512 * c:512 * (c + 1)],
                    start=True,
                    stop=True,
                    tile_position=(64 * h, 0),
                )
            pt = work.tile([128, 1024], BF16, tag="pt", bufs=8, name="pt")
            nc.scalar.activation(
                pt[:], stp[:], mybir.ActivationFunctionType.Exp, scale=SCALE
            )
            pending.append({"ot": ot, "p": p, "c": c, "mt": mt, "pt": pt})
            if len(pending) == 4:
                flush_one()
            if filler is not None:
                filler(c, mt)

    # ---- Emission schedule ----
    # Phase B: dummy matmuls cover the initial DMA window (wqkv + xT chunk
    # 0 land ~8us in), then Kt pair 0 per xT-chunk arrival (the critical
    # path to attention start), Qt(p0, c0), and the first two V tiles.
    # Dummies bridge the xT chunk arrival gaps (the PE would otherwise
    # idle on the DMA semaphore and trip the HAM clock throttle).
    # Attention starts right after Kt(p0) chunks 0-1; chunks 2-3 stream in
    # as chunk-0 fillers, arrival-matched to their St consumers.
    phase_b = [
        ("dummy", 34),
        ("qk", "k", 0, 0), ("qk", "q", 0, 0), ("v", 0), ("v", 1),
        ("dummy", 24), ("qk", "k", 0, 1),
    ]
    for u in phase_b:
        emit_unit(u)

    # Fillers (slot = mt index, NT = after last exp).  Fillers sit at the
    # START of each chunk: the first AV of a chunk waits ~1.2us for the
    # previous chunk's ot evacuation, and St(mt=1) waits for the previous
    # chunk's last exp to free its stp slot — front-loaded fillers absorb
    # both so the PE never idles (an idle PE trips the HAM clock throttle).
    # NOTE: inside emit_att_chunk the AV flush for key tiles (mt-2, mt-1)
    # is emitted BEFORE the slot-mt filler, so V tile j must sit at slot
    # <= j+1 (one earlier to cover the DVE evac latency).
    att0_fill = {
        0: {0: [("v", 2), ("v", 3)], 1: [("v", 4), ("v", 5)],
            2: [("v", 6), ("v", 7)], 3: [("qk", "k", 0, 2)],
            5: [("v", 8), ("v", 9)], 6: [("v", 10)],
            7: [("qk", "k", 0, 3)], 9: [("v", 11), ("v", 12)],
            11: [("v", 13), ("v", 14)], 13: [("v", 15), ("qk", "q", 0, 1)]},
        1: {0: [("qk", "k", 1, 0)], 1: [("qk", "q", 0, 2)],
            8: [("qk", "k", 1, 1)]},
        2: {0: [("qk", "k", 1, 2)], 1: [("qk", "q", 0, 3)],
            8: [("qk", "q", 1, 0)]},
        3: {0: [("qk", "k", 1, 3)], 8: [("qk", "q", 1, 1)]},
    }

    def att0_filler(c, mt):
        for u in att0_fill[c].get(mt, ()):
            emit_unit(u)

    for c in range(CH):
        emit_att_chunk(0, c, filler=att0_filler)

    # Phase D: attention pair 1.  Chunk 0 finishes the last Qt unit;
    # chunks 1-3 carry the out-projection for the query rows of chunk c-1
    # (complete for both pairs by then), front-loaded for the same reason.
    att1_fill = {
        0: {0: [("qk", "q", 1, 2)], 8: [("qk", "q", 1, 3)]},
    }
    # proj fillers start at slot 2: the previous chunk's normalize is only
    # emitted during this chunk's slot-1 flush, and proj reads its o rows.
    for c in range(1, CH):
        units = [("proj", nt, jc)
                 for nt in range(4 * (c - 1), 4 * c) for jc in range(2)]
        slots = [2, 2, 3, 3, 4, 6, 8, 10]
        att1_fill[c] = {}
        for s, u in zip(slots, units):
            att1_fill[c].setdefault(s, []).append(u)

    def att1_filler(c, mt):
        for u in att1_fill.get(c, {}).get(mt, ()):
            emit_unit(u)

    for c in range(CH):
        emit_att_chunk(1, c, filler=att1_filler)
    # Drain the carried AVs of the last chunk; its normalize copies go to
    # ScalarE (idle after the last exp) so the DVE backlog of proj
    # evacuations never delays the final proj units.
    while pending:
        flush_one(last_norm_engine="scalar")

    # Drain: last four row tiles.  Alternate the PSUM evacuations between
    # ScalarE and DVE to halve the tail.
    i = 0
    for nt in range(12, 16):
        for jc in range(2):
            emit_proj_unit(nt, jc, evac="scalar" if i % 2 else None)
            i += 1

    ctx.close()


def _build():
    global _CACHED_NC
    if _CACHED_NC is not None:
        return _CACHED_NC
    nc = bacc.Bacc(
        "TRN2",
        target_bir_lowering=False,
        debug=False,
        enable_asserts=True,
        num_devices=N_CORES,
    )
    xt_d = nc.dram_tensor("xt", [DIM, N], BF16, kind="ExternalInput").ap()
    wqkv_d = nc.dram_tensor("wqkv", [DIM, 768], BF16, kind="ExternalInput").ap()
    wo_d = nc.dram_tensor("wo", [256, DIM], BF16, kind="ExternalInput").ap()
    out_d = nc.dram_tensor("out", [N, DIM], BF16, kind="ExternalOutput").ap()

    with tile.TileContext(nc) as tc:
        _emit_kernel(tc, xt_d, wqkv_d, wo_d, out_d)
    nc.compile()
    _CACHED_NC = nc
    return nc


def _in_maps(x, w_qkv, w_out):
    import ml_dtypes

    bf = ml_dtypes.bfloat16
    maps = []
    for c in range(N_CORES):
        b, g = divmod(c, 4)
        cols = slice(256 * g, 256 * (g + 1))
        wqkv_c = np.ascontiguousarray(
            np.concatenate(
                [
                    w_qkv[:, cols],
                    w_qkv[:, INNER:][:, cols],
                    w_qkv[:, 2 * INNER:][:, cols],
                ],
                axis=1,
            ).astype(bf)
        )
        maps.append(
            {
                "xt": np.ascontiguousarray(x[b].T.astype(bf)),
                "wqkv": wqkv_c,
                "wo": np.ascontiguousarray(w_out[cols, :].astype(bf)),
            }
        )
    return maps


def _run(x, w_qkv, w_out, b_out, trace=False):
    nc = _build()
    res = run_bass_kernel_spmd(
        nc, _in_maps(x, w_qkv, w_out), list(range(N_CORES)), trace=trace
    )
    partials = np.stack(
        [np.asarray(res.results[c]["out"], dtype=np.float32)
         for c in range(N_CORES)]
    )
    out = np.empty((B, N, DIM), dtype=np.float32)
    for b in range(B):
        out[b] = partials[4 * b:4 * b + 4].sum(axis=0) + b_out
    return out, res


def kernel(x, w_qkv, w_out, b_out):
    out, _ = _run(
        np.asarray(x, dtype=np.float32),
        np.asarray(w_qkv, dtype=np.float32),
        np.asarray(w_out, dtype=np.float32),
        np.asarray(b_out, dtype=np.float32),
    )
    return out


# revision 33
# speedup vs baseline: 1.0350x; 1.0254x over previous
"""Multi-head attention (B=2, N=2048, dim=1024, heads=16, dim_head=64) on
8 TRN2 NeuronCores.

Sharding: data-parallel over batch (2) x tensor-parallel over heads (4 per
core).  Core c handles batch b = c//4 and heads [4g, 4g+4), g = c%4.  Each
core computes its 4 heads' attention plus the partial out-projection
(O_heads @ w_out[head rows]); the host sums the 4 partials per batch and
adds the bias.

Per-core device algorithm (bf16 matmul inputs, fp32 PSUM accumulate; exp
without max-subtraction — scores are ~N(0,1) so exp never overflows):
  xT  [1024, 2048] = x[b].T               (transposed on host, free)
  Qt/Kt [128(i of head pair), 2048(n)] = w_slice.T @ xT    (W stationary)
  V   [128(m), 16(mt), 4(h), 65]  natural layout, col 64 = ones so the
       attention-value matmul also produces the softmax denominator.
  per head pair (A,B packed in PE rows 0-63 / 64-127 via tile_position),
  per 512-wide query chunk, per key tile mt:
       St[h] = Kt_h[64, 128].T @ Qt_h[64, 512] -> PSUM [128, 1024] (h0|h1
               in two adjacent banks of one wide tile)
       Pt = exp(St * 1/8)                      -> one [128, 1024] ScalarE
               activation covering both heads (halves ACT instruction count)
       Ot' += V'[128, 65].T @ Pt[:, 512h:+512] -> PSUM [65, 512] accum
  normalize: o = Ot'[0:64] * (1 / Ot'[64]) (DVE recip + GPSIMD bcast + DVE)
  proj: out[nt*128:+128, jc*512:+512] = sum_p o_sb[:,p,nt].T @ wo[:,p,jc]

Schedule: input DMAs ordered wqkv, xT chunk 0..3, wo; QKV projection units
emitted per-chunk as the xT chunks land (dummy matmuls bridge the initial
DMA-only window so the HAM clock gate never throttles); attention pair 0
starts as soon as Kt pair 0 is complete (~20us), with the remaining QKV
units as fillers; attention pair 1 carries the out-projection as filler.
"""
import numpy as np

import concourse.bass as bass
import concourse.mybir as mybir
import concourse.tile as tile
from concourse import bacc
from concourse.bass_utils import run_bass_kernel_spmd

# Problem constants (hardcoded per contract).
B = 2
N = 2048
DIM = 1024
HEADS = 16
DH = 64
INNER = HEADS * DH
SCALE = DH ** -0.5

N_CORES = 8
HEADS_PER_CORE = 4
PAIRS = 2          # head pairs per core
NT = N // 128      # 16 key/query tiles
DT = DIM // 128    # 8 contraction tiles
CH = N // 512      # 4 query chunks
F32 = mybir.dt.float32
F32R = mybir.dt.float32r
BF16 = mybir.dt.bfloat16

_CACHED_NC = None


def _emit_kernel(tc, xt_d, wqkv_d, wo_d, out_d):
    nc = tc.nc

    from contextlib import ExitStack

    ctx = ExitStack()
    per = ctx.enter_context(tc.tile_pool(name="persist", bufs=1))
    psum = ctx.enter_context(tc.tile_pool(name="psum", bufs=1, space="PSUM"))
    work = ctx.enter_context(tc.tile_pool(name="work", bufs=1))

    # Persistent SBUF tensors.
    xt_sb = per.tile([128, DT, N], BF16, tag="xt")
    wqkv_sb = per.tile([128, DT, 768], BF16, tag="wqkv")
    wo_sb = per.tile([128, PAIRS, DIM], BF16, tag="wo")
    qt_sb = per.tile([128, PAIRS, N], BF16, tag="qt")
    kt_sb = per.tile([128, PAIRS, N], BF16, tag="kt")
    v_sb = per.tile([128, NT, HEADS_PER_CORE, DH + 1], BF16, tag="v")
    o_sb = per.tile([128, PAIRS, N], BF16, tag="o")

    # Input DMAs in consumption order: wqkv (needed by every projection
    # unit), xT chunk by chunk, wo last (first used ~100us in).  One wide
    # DMA per tensor/chunk — a single instruction's packets already spread
    # over all DMA engines, and fewer issues (6 vs 24) gets every transfer
    # in flight within ~4us of kernel start.
    # Input DMAs: each ring allows only ~4 outstanding ops, so in-flight
    # bytes scale with op size — use full-width per-dt ops (0.5 MB for xT)
    # interleaved across the two HWDGE rings (Sync + ScalarE, idle until
    # the first exp).  A wide rearranged 3D DMA decodes wrong on the
    # hardware DGE, so ops stay plain 2D.
    for dt in range(DT):
        eng = nc.sync if dt % 2 == 0 else nc.scalar
        eng.dma_start(wqkv_sb[:, dt, :], wqkv_d[128 * dt:128 * (dt + 1), :])
    for dt in range(DT):
        eng = nc.sync if dt % 2 == 0 else nc.scalar
        eng.dma_start(xt_sb[:, dt, :], xt_d[128 * dt:128 * (dt + 1), :])
    for p in range(PAIRS):
        nc.scalar.dma_start(wo_sb[:, p, :], wo_d[128 * p:128 * (p + 1), :])

    # Ones column of V' (gives the softmax denominator through the AV matmul).
    ones_sb = per.tile([128, NT * HEADS_PER_CORE], F32, tag="ones")
    nc.vector.memset(ones_sb[:], 1.0)
    nc.vector.tensor_copy(
        v_sb[:, :, :, DH:DH + 1],
        ones_sb[:].rearrange("p (a b c) -> p a b c", b=HEADS_PER_CORE, c=1),
    )
    # Touch Exp once so the ACT table DMA (~1.3us + pseudo-load) happens
    # during the startup phase rather than before the first real exp.
    warm = work.tile([1, 1], F32, tag="warm")
    nc.scalar.activation(
        warm[:], ones_sb[0:1, 0:1], mybir.ActivationFunctionType.Exp, scale=1.0
    )

    def emit_dummies(n):
        # Keep the PE busy through DMA-only windows so the HAM clock gate
        # stays at full speed (an idle PE gets throttled to 1.2 GHz for
        # several us).  Output is never read.
        for _ in range(n):
            dmy = psum.tile([64, 64], F32, tag="stp", bufs=2, name="dummy")
            nc.tensor.matmul(
                dmy[:], ones_sb[:, 0:64], ones_sb[:, 0:64], start=True, stop=True
            )

    def emit_qk_chunk(which, p, c):
        """Qt or Kt for head pair p, n-chunk c: [128, 512] of W.T @ xT."""
        src = qt_sb if which == "q" else kt_sb
        col0 = (0 if which == "q" else 256) + 128 * p
        ps = psum.tile([128, 512], F32, tag="qk", bufs=2)
        for dt in range(DT):
            nc.tensor.matmul(
                ps[:],
                wqkv_sb[:, dt, col0:col0 + 128],
                xt_sb[:, dt, 512 * c:512 * (c + 1)],
                start=(dt == 0),
                stop=(dt == DT - 1),
            )
        nc.vector.tensor_copy(src[:, p, 512 * c:512 * (c + 1)], ps[:])

    def emit_v_tile(mt):
        """V natural [128(m), 256(4 heads x 64)] for key tile mt."""
        ps = psum.tile([128, 256], F32, tag="qk", bufs=2)
        for dt in range(DT):
            nc.tensor.matmul(
                ps[:],
                xt_sb[:, dt, 128 * mt:128 * (mt + 1)],
                wqkv_sb[:, dt, 512:768],
                start=(dt == 0),
                stop=(dt == DT - 1),
            )
        nc.vector.tensor_copy(
            v_sb[:, mt, :, 0:DH],
            ps[:].rearrange("p (h d) -> p h d", h=HEADS_PER_CORE),
        )

    ev_tiles = {}

    def emit_proj_unit(nt, jc, evac=None):
        """out[128nt:+128, 512jc:+512] = sum_p o_sb[:,p,nt].T @ wo[:,p,jc].

        Both jc halves of a row tile land in one [128, 1024] bf16 staging
        tile; the output DMA is split in two ops on two issuing engines so
        the tail transfer overlaps itself across DMA rings.
        """
        if nt not in ev_tiles:
            ev_tiles[nt] = work.tile([128, DIM], BF16, tag="ev", bufs=4, name="ev")
        ev = ev_tiles[nt]
        ps = psum.tile([128, 512], F32, tag="qk", bufs=2)
        for p in range(PAIRS):
            nc.tensor.matmul(
                ps[:],
                o_sb[:, p, 128 * nt:128 * (nt + 1)],
                wo_sb[:, p, 512 * jc:512 * (jc + 1)],
                start=(p == 0),
                stop=(p == PAIRS - 1),
            )
        if evac == "scalar":
            nc.scalar.copy(ev[:, 512 * jc:512 * (jc + 1)], ps[:])
        else:
            nc.vector.tensor_copy(ev[:, 512 * jc:512 * (jc + 1)], ps[:])
        if jc == 1:
            r0 = 128 * nt
            if evac is None:
                nc.sync.dma_start(out_d[r0:r0 + 128, :], ev[:])
            else:
                # Drain: split by rows across both HWDGE engines so the
                # final transfers overlap across DMA rings.
                nc.sync.dma_start(out_d[r0:r0 + 64, :], ev[0:64, :])
                nc.scalar.dma_start(out_d[r0 + 64:r0 + 128, :], ev[64:128, :])
            del ev_tiles[nt]

    def emit_unit(u):
        if u[0] == "v":
            emit_v_tile(u[1])
        elif u[0] == "qk":
            emit_qk_chunk(u[1], u[2], u[3])
        elif u[0] == "proj":
            emit_proj_unit(u[1], u[2])
        elif u[0] == "dummy":
            emit_dummies(u[1])

    def emit_normalize(ot, p, c, engine="vector"):
        """Normalize: o = Ot'[0:64] / Ot'[64].  First evacuate each head's
        Ot' to SBUF (denominator row to a partition-0 tile, numerator via
        one wide copy) — that frees the ot PSUM bank after ~1.1us so the
        next chunk's first AV matmul never waits on the rest of the chain
        (recip/broadcast/mul).  engine="scalar" moves the copies to the
        otherwise-idle ScalarE during the drain.

        (reciprocal_approx_fast misbehaves on hardware when its input AP
        sits at a nonzero base partition, so den gets a partition-0 copy.)
        """
        copy = (nc.scalar.copy if engine == "scalar"
                else lambda o, i: nc.vector.tensor_copy(o, i))
        den, otmp, recip, rbc = ([None, None] for _ in range(4))
        for h in range(2):
            den[h] = work.tile([1, 512], F32, tag="den", bufs=4, name=f"den{h}")
            copy(den[h][:], ot[h][DH:DH + 1, :])
            otmp[h] = work.tile([DH, 512], F32, tag="otmp", bufs=4, name=f"otmp{h}")
            copy(otmp[h][:], ot[h][0:DH, :])
        for h in range(2):
            recip[h] = work.tile([1, 512], F32, tag="recip", bufs=4, name=f"rec{h}")
            nc.vector.reciprocal_approx_fast(recip[h][:], den[h][:])
        for h in range(2):
            rbc[h] = work.tile([64, 512], F32, tag="rbc", bufs=4, name=f"rbc{h}")
            nc.gpsimd.partition_broadcast(rbc[h][:], recip[h][:])
        for h in range(2):
            nc.vector.tensor_mul(
                o_sb[64 * h:64 * (h + 1), p, 512 * c:512 * (c + 1)],
                otmp[h][:],
                rbc[h][:],
            )

    # AV matmuls lag the St/exp of the current key tile by two slots so the
    # in-order PE queue never head-of-line blocks on ScalarE.  The pending
    # list carries across chunk boundaries (software pipelining): a chunk's
    # last two AVs and its normalize are emitted during the next chunk's
    # first two slots, by which time its last exp has certainly finished.
    pending = []

    def flush_one(last_norm_engine="vector"):
        e = pending.pop(0)
        for h in range(2):
            nc.tensor.matmul(
                e["ot"][h][:],
                v_sb[:, e["mt"], 2 * e["p"] + h, :],
                e["pt"][:, 512 * h:512 * (h + 1)],
                start=(e["mt"] == 0),
                stop=(e["mt"] == NT - 1),
            )
        if e["mt"] == NT - 1:
            emit_normalize(e["ot"], e["p"], e["c"], engine=last_norm_engine)

    def emit_att_chunk(p, c, filler=None):
        """Attention for head pair p, query chunk c (cols 512c..512c+512).

        The two heads of a pair sit in PE rows 0-63 / 64-127 (tile_position
        row packing); their St outputs land in the two adjacent PSUM banks
        of one [128, 1024] tile so a single wide ScalarE activation
        exponentiates both.
        """
        ot = [
            psum.tile([DH + 1, 512], F32, tag="ot", bufs=2, name=f"ot{h}")
            for h in range(2)
        ]
        for mt in range(NT):
            stp = psum.tile([128, 1024], F32, tag="stp", bufs=2, name="stp")
            for h in range(2):
                nc.tensor.matmul(
                    stp[:, 512 * h:512 * (h + 1)],
                    kt_sb[64 * h:64 * (h + 1), p, 128 * mt:128 * (mt + 1)],
                    qt_sb[64 * h:64 * (h + 1), p, 512 * c:512 * (c + 1)],
                    start=True,
                    stop=True,
                    tile_position=(64 * h, 0),
                )
            pt = work.tile([128, 1024], BF16, tag="pt", bufs=8, name="pt")
            nc.scalar.activation(
                pt[:], stp[:], mybir.ActivationFunctionType.Exp, scale=SCALE
            )
            pending.append({"ot": ot, "p": p, "c": c, "mt": mt, "pt": pt})
            if len(pending) == 4:
                flush_one()
            if filler is not None:
                filler(c, mt)

    # ---- Emission schedule ----
    # Phase B: dummy matmuls cover the initial DMA window (wqkv + xT chunk
    # 0 land ~8us in), then Kt pair 0 per xT-chunk arrival (the critical
    # path to attention start), Qt(p0, c0), and the first two V tiles.
    # Dummies bridge the xT chunk arrival gaps (the PE would otherwise
    # idle on the DMA semaphore and trip the HAM clock throttle).
    # Attention starts right after Kt(p0) chunks 0-1; chunks 2-3 stream in
    # as chunk-0 fillers, arrival-matched to their St consumers.
    phase_b = [
        ("dummy", 52),
        ("qk", "k", 0, 0), ("qk", "q", 0, 0), ("v", 0), ("v", 1),
        ("qk", "k", 0, 1),
    ]
    for u in phase_b:
        emit_unit(u)

    # Fillers (slot = mt index, NT = after last exp).  Fillers sit at the
    # START of each chunk: the first AV of a chunk waits ~1.2us for the
    # previous chunk's ot evacuation, and St(mt=1) waits for the previous
    # chunk's last exp to free its stp slot — front-loaded fillers absorb
    # both so the PE never idles (an idle PE trips the HAM clock throttle).
    # NOTE: inside emit_att_chunk the AV flush for key tiles (mt-2, mt-1)
    # is emitted BEFORE the slot-mt filler, so V tile j must sit at slot
    # <= j+1 (one earlier to cover the DVE evac latency).
    att0_fill = {
        0: {0: [("v", 2), ("v", 3)], 1: [("v", 4), ("v", 5)],
            2: [("v", 6), ("v", 7)], 3: [("qk", "k", 0, 2)],
            5: [("v", 8), ("v", 9)], 6: [("v", 10)],
            7: [("qk", "k", 0, 3)], 9: [("v", 11), ("v", 12)],
            11: [("v", 13), ("v", 14)], 13: [("v", 15), ("qk", "q", 0, 1)]},
        1: {0: [("qk", "k", 1, 0)], 1: [("qk", "q", 0, 2)],
            8: [("qk", "k", 1, 1)]},
        2: {0: [("qk", "k", 1, 2)], 1: [("qk", "q", 0, 3)],
            8: [("qk", "q", 1, 0)]},
        3: {0: [("qk", "k", 1, 3)], 8: [("qk", "q", 1, 1)]},
    }

    def att0_filler(c, mt):
        for u in att0_fill[c].get(mt, ()):
            emit_unit(u)

    for c in range(CH):
        emit_att_chunk(0, c, filler=att0_filler)

    # Phase D: attention pair 1.  Chunk 0 finishes the last Qt unit;
    # chunks 1-3 carry the out-projection for the query rows of chunk c-1
    # (complete for both pairs by then), front-loaded for the same reason.
    att1_fill = {
        0: {0: [("qk", "q", 1, 2)], 8: [("qk", "q", 1, 3)]},
    }
    # proj fillers start at slot 3: the previous chunk's normalize is only
    # emitted during this chunk's slot-2 flush, and proj reads its o rows.
    # Slots 0-2 already carry the previous chunk's final AVs + normalize,
    # so proj spreads over the middle to keep the exp stream dense.
    for c in range(1, CH):
        units = [("proj", nt, jc)
                 for nt in range(4 * (c - 1), 4 * c) for jc in range(2)]
        slots = [3, 5, 7, 9, 10, 11, 12, 13]
        att1_fill[c] = {}
        for s, u in zip(slots, units):
            att1_fill[c].setdefault(s, []).append(u)

    def att1_filler(c, mt):
        for u in att1_fill.get(c, {}).get(mt, ()):
            emit_unit(u)

    for c in range(CH):
        emit_att_chunk(1, c, filler=att1_filler)
    # Drain the carried AVs of the last chunk; its normalize copies go to
    # ScalarE (idle after the last exp) so the DVE backlog of proj
    # evacuations never delays the final proj units.
    while pending:
        flush_one(last_norm_engine="scalar")

    # Drain: last four row tiles.  Alternate the PSUM evacuations between
    # ScalarE and DVE to halve the tail.
    i = 0
    for nt in range(12, 16):
        for jc in range(2):
            emit_proj_unit(nt, jc, evac="scalar" if i % 2 else None)
            i += 1

    ctx.close()


def _build():
    global _CACHED_NC
    if _CACHED_NC is not None:
        return _CACHED_NC
    nc = bacc.Bacc(
        "TRN2",
        target_bir_lowering=False,
        debug=False,
        enable_asserts=True,
        num_devices=N_CORES,
    )
    xt_d = nc.dram_tensor("xt", [DIM, N], BF16, kind="ExternalInput").ap()
    wqkv_d = nc.dram_tensor("wqkv", [DIM, 768], BF16, kind="ExternalInput").ap()
    wo_d = nc.dram_tensor("wo", [256, DIM], BF16, kind="ExternalInput").ap()
    out_d = nc.dram_tensor("out", [N, DIM], BF16, kind="ExternalOutput").ap()

    with tile.TileContext(nc) as tc:
        _emit_kernel(tc, xt_d, wqkv_d, wo_d, out_d)
    nc.compile()
    _CACHED_NC = nc
    return nc


def _in_maps(x, w_qkv, w_out):
    import ml_dtypes

    bf = ml_dtypes.bfloat16
    maps = []
    for c in range(N_CORES):
        b, g = divmod(c, 4)
        cols = slice(256 * g, 256 * (g + 1))
        wqkv_c = np.ascontiguousarray(
            np.concatenate(
                [
                    w_qkv[:, cols],
                    w_qkv[:, INNER:][:, cols],
                    w_qkv[:, 2 * INNER:][:, cols],
                ],
                axis=1,
            ).astype(bf)
        )
        maps.append(
            {
                "xt": np.ascontiguousarray(x[b].T.astype(bf)),
                "wqkv": wqkv_c,
                "wo": np.ascontiguousarray(w_out[cols, :].astype(bf)),
            }
        )
    return maps


def _run(x, w_qkv, w_out, b_out, trace=False):
    nc = _build()
    res = run_bass_kernel_spmd(
        nc, _in_maps(x, w_qkv, w_out), list(range(N_CORES)), trace=trace
    )
    partials = np.stack(
        [np.asarray(res.results[c]["out"], dtype=np.float32)
         for c in range(N_CORES)]
    )
    out = np.empty((B, N, DIM), dtype=np.float32)
    for b in range(B):
        out[b] = partials[4 * b:4 * b + 4].sum(axis=0) + b_out
    return out, res


def kernel(x, w_qkv, w_out, b_out):
    out, _ = _run(
        np.asarray(x, dtype=np.float32),
        np.asarray(w_qkv, dtype=np.float32),
        np.asarray(w_out, dtype=np.float32),
        np.asarray(b_out, dtype=np.float32),
    )
    return out


# revision 38
# speedup vs baseline: 1.0515x; 1.0160x over previous
"""Multi-head attention (B=2, N=2048, dim=1024, heads=16, dim_head=64) on
8 TRN2 NeuronCores.

Sharding: data-parallel over batch (2) x tensor-parallel over heads (4 per
core).  Core c handles batch b = c//4 and heads [4g, 4g+4), g = c%4.  Each
core computes its 4 heads' attention plus the partial out-projection
(O_heads @ w_out[head rows]); the host sums the 4 partials per batch and
adds the bias.

Per-core device algorithm (bf16 matmul inputs, fp32 PSUM accumulate; exp
without max-subtraction — scores are ~N(0,1) so exp never overflows):
  xT  [1024, 2048] = x[b].T               (transposed on host, free)
  Qt/Kt [128(i of head pair), 2048(n)] = w_slice.T @ xT    (W stationary)
  V   [128(m), 16(mt), 4(h), 65]  natural layout, col 64 = ones so the
       attention-value matmul also produces the softmax denominator.
  per head pair (A,B packed in PE rows 0-63 / 64-127 via tile_position),
  per 512-wide query chunk, per key tile mt:
       St[h] = Kt_h[64, 128].T @ Qt_h[64, 512] -> PSUM [128, 1024] (h0|h1
               in two adjacent banks of one wide tile)
       Pt = exp(St * 1/8)                      -> one [128, 1024] ScalarE
               activation covering both heads (halves ACT instruction count)
       Ot' += V'[128, 65].T @ Pt[:, 512h:+512] -> PSUM [65, 512] accum
  normalize: o = Ot'[0:64] * (1 / Ot'[64]) (DVE recip + GPSIMD bcast + DVE)
  proj: out[nt*128:+128, jc*512:+512] = sum_p o_sb[:,p,nt].T @ wo[:,p,jc]

Schedule: input DMAs ordered wqkv, xT chunk 0..3, wo; QKV projection units
emitted per-chunk as the xT chunks land (dummy matmuls bridge the initial
DMA-only window so the HAM clock gate never throttles); attention pair 0
starts as soon as Kt pair 0 is complete (~20us), with the remaining QKV
units as fillers; attention pair 1 carries the out-projection as filler.
"""
import numpy as np

import concourse.bass as bass
import concourse.mybir as mybir
import concourse.tile as tile
from concourse import bacc
from concourse.bass_utils import run_bass_kernel_spmd

# Problem constants (hardcoded per contract).
B = 2
N = 2048
DIM = 1024
HEADS = 16
DH = 64
INNER = HEADS * DH
SCALE = DH ** -0.5

N_CORES = 8
HEADS_PER_CORE = 4
PAIRS = 2          # head pairs per core
NT = N // 128      # 16 key/query tiles
DT = DIM // 128    # 8 contraction tiles
CH = N // 512      # 4 query chunks
F32 = mybir.dt.float32
F32R = mybir.dt.float32r
BF16 = mybir.dt.bfloat16

_CACHED_NC = None


def _emit_kernel(tc, xt_d, wqkv_d, wo_d, out_d):
    nc = tc.nc

    from contextlib import ExitStack

    ctx = ExitStack()
    per = ctx.enter_context(tc.tile_pool(name="persist", bufs=1))
    psum = ctx.enter_context(tc.tile_pool(name="psum", bufs=1, space="PSUM"))
    work = ctx.enter_context(tc.tile_pool(name="work", bufs=1))

    # Persistent SBUF tensors.
    xt_sb = per.tile([128, DT, N], BF16, tag="xt")
    wqkv_sb = per.tile([128, DT, 768], BF16, tag="wqkv")
    wo_sb = per.tile([128, PAIRS, DIM], BF16, tag="wo")
    qt_sb = per.tile([128, PAIRS, N], BF16, tag="qt")
    kt_sb = per.tile([128, PAIRS, N], BF16, tag="kt")
    v_sb = per.tile([128, NT, HEADS_PER_CORE, DH + 1], BF16, tag="v")
    o_sb = per.tile([128, PAIRS, N], BF16, tag="o")

    # Input DMAs in consumption order: wqkv (needed by every projection
    # unit), xT chunk by chunk, wo last (first used ~100us in).  One wide
    # DMA per tensor/chunk — a single instruction's packets already spread
    # over all DMA engines, and fewer issues (6 vs 24) gets every transfer
    # in flight within ~4us of kernel start.
    # Input DMAs: each ring allows only ~4 outstanding ops, so in-flight
    # bytes scale with op size — use full-width per-dt ops (0.5 MB for xT)
    # interleaved across the two HWDGE rings (Sync + ScalarE, idle until
    # the first exp).  A wide rearranged 3D DMA decodes wrong on the
    # hardware DGE, so ops stay plain 2D.
    for dt in range(DT):
        eng = nc.sync if dt % 2 == 0 else nc.scalar
        eng.dma_start(wqkv_sb[:, dt, :], wqkv_d[128 * dt:128 * (dt + 1), :])
    for half in range(2):
        for dt in range(DT):
            eng = nc.sync if dt % 2 == 0 else nc.scalar
            cols = slice(1024 * half, 1024 * (half + 1))
            eng.dma_start(xt_sb[:, dt, cols], xt_d[128 * dt:128 * (dt + 1), cols])
    for p in range(PAIRS):
        nc.scalar.dma_start(wo_sb[:, p, :], wo_d[128 * p:128 * (p + 1), :])

    # Ones column of V' (gives the softmax denominator through the AV matmul).
    ones_sb = per.tile([128, NT * HEADS_PER_CORE], F32, tag="ones")
    nc.vector.memset(ones_sb[:], 1.0)
    nc.vector.tensor_copy(
        v_sb[:, :, :, DH:DH + 1],
        ones_sb[:].rearrange("p (a b c) -> p a b c", b=HEADS_PER_CORE, c=1),
    )
    # Touch Exp once so the ACT table DMA (~1.3us + pseudo-load) happens
    # during the startup phase rather than before the first real exp.
    warm = work.tile([1, 1], F32, tag="warm")
    nc.scalar.activation(
        warm[:], ones_sb[0:1, 0:1], mybir.ActivationFunctionType.Exp, scale=1.0
    )

    def emit_dummies(n):
        # Keep the PE busy through DMA-only windows so the HAM clock gate
        # stays at full speed (an idle PE gets throttled to 1.2 GHz for
        # several us).  Output is never read.
        for _ in range(n):
            dmy = psum.tile([64, 64], F32, tag="stp", bufs=2, name="dummy")
            nc.tensor.matmul(
                dmy[:], ones_sb[:, 0:64], ones_sb[:, 0:64], start=True, stop=True
            )

    def emit_qk_chunk(which, p, c):
        """Qt or Kt for head pair p, n-chunk c: [128, 512] of W.T @ xT."""
        src = qt_sb if which == "q" else kt_sb
        col0 = (0 if which == "q" else 256) + 128 * p
        ps = psum.tile([128, 512], F32, tag="qk", bufs=2)
        for dt in range(DT):
            nc.tensor.matmul(
                ps[:],
                wqkv_sb[:, dt, col0:col0 + 128],
                xt_sb[:, dt, 512 * c:512 * (c + 1)],
                start=(dt == 0),
                stop=(dt == DT - 1),
            )
        nc.vector.tensor_copy(src[:, p, 512 * c:512 * (c + 1)], ps[:])

    def emit_v_tile(mt):
        """V natural [128(m), 256(4 heads x 64)] for key tile mt."""
        ps = psum.tile([128, 256], F32, tag="qk", bufs=2)
        for dt in range(DT):
            nc.tensor.matmul(
                ps[:],
                xt_sb[:, dt, 128 * mt:128 * (mt + 1)],
                wqkv_sb[:, dt, 512:768],
                start=(dt == 0),
                stop=(dt == DT - 1),
            )
        nc.vector.tensor_copy(
            v_sb[:, mt, :, 0:DH],
            ps[:].rearrange("p (h d) -> p h d", h=HEADS_PER_CORE),
        )

    ev_tiles = {}

    def emit_proj_unit(nt, jc, evac=None, psum_tag="qk"):
        """out[128nt:+128, 512jc:+512] = sum_p o_sb[:,p,nt].T @ wo[:,p,jc].

        Both jc halves of a row tile land in one [128, 1024] bf16 staging
        tile; the output DMA is split in two ops on two issuing engines so
        the tail transfer overlaps itself across DMA rings.
        """
        if nt not in ev_tiles:
            ev_tiles[nt] = work.tile([128, DIM], BF16, tag="ev", bufs=4, name="ev")
        ev = ev_tiles[nt]
        ps = psum.tile([128, 512], F32, tag=psum_tag, bufs=2, name="ps")
        for p in range(PAIRS):
            nc.tensor.matmul(
                ps[:],
                o_sb[:, p, 128 * nt:128 * (nt + 1)],
                wo_sb[:, p, 512 * jc:512 * (jc + 1)],
                start=(p == 0),
                stop=(p == PAIRS - 1),
            )
        if evac == "scalar":
            nc.scalar.copy(ev[:, 512 * jc:512 * (jc + 1)], ps[:])
        else:
            nc.vector.tensor_copy(ev[:, 512 * jc:512 * (jc + 1)], ps[:])
        if jc == 1:
            r0 = 128 * nt
            if evac is None:
                nc.sync.dma_start(out_d[r0:r0 + 128, :], ev[:])
            else:
                # Drain: split by rows across both HWDGE engines so the
                # final transfers overlap across DMA rings.
                nc.sync.dma_start(out_d[r0:r0 + 64, :], ev[0:64, :])
                nc.scalar.dma_start(out_d[r0 + 64:r0 + 128, :], ev[64:128, :])
            del ev_tiles[nt]

    def emit_unit(u):
        if u[0] == "v":
            emit_v_tile(u[1])
        elif u[0] == "qk":
            emit_qk_chunk(u[1], u[2], u[3])
        elif u[0] == "proj":
            emit_proj_unit(u[1], u[2])
        elif u[0] == "dummy":
            emit_dummies(u[1])

    def emit_normalize(ot, p, c, engine="vector"):
        """Normalize: o = Ot'[0:64] / Ot'[64].  First evacuate each head's
        Ot' to SBUF (denominator row to a partition-0 tile, numerator via
        one wide copy) — that frees the ot PSUM bank after ~1.1us so the
        next chunk's first AV matmul never waits on the rest of the chain
        (recip/broadcast/mul).  engine="scalar" moves the copies to the
        otherwise-idle ScalarE during the drain.

        (reciprocal_approx_fast misbehaves on hardware when its input AP
        sits at a nonzero base partition, so den gets a partition-0 copy.)
        """
        copy = (nc.scalar.copy if engine == "scalar"
                else lambda o, i: nc.vector.tensor_copy(o, i))
        den, otmp, recip, rbc = ([None, None] for _ in range(4))
        for h in range(2):
            den[h] = work.tile([1, 512], F32, tag="den", bufs=4, name=f"den{h}")
            copy(den[h][:], ot[h][DH:DH + 1, :])
            otmp[h] = work.tile([DH, 512], F32, tag="otmp", bufs=4, name=f"otmp{h}")
            copy(otmp[h][:], ot[h][0:DH, :])
        for h in range(2):
            recip[h] = work.tile([1, 512], F32, tag="recip", bufs=4, name=f"rec{h}")
            nc.vector.reciprocal_approx_fast(recip[h][:], den[h][:])
        for h in range(2):
            rbc[h] = work.tile([64, 512], F32, tag="rbc", bufs=4, name=f"rbc{h}")
            nc.gpsimd.partition_broadcast(rbc[h][:], recip[h][:])
        for h in range(2):
            nc.vector.tensor_mul(
                o_sb[64 * h:64 * (h + 1), p, 512 * c:512 * (c + 1)],
                otmp[h][:],
                rbc[h][:],
            )

    # AV matmuls lag the St/exp of the current key tile by two slots so the
    # in-order PE queue never head-of-line blocks on ScalarE.  The pending
    # list carries across chunk boundaries (software pipelining): a chunk's
    # last two AVs and its normalize are emitted during the next chunk's
    # first two slots, by which time its last exp has certainly finished.
    pending = []

    def flush_one(last_norm_engine="vector"):
        e = pending.pop(0)
        for h in range(2):
            nc.tensor.matmul(
                e["ot"][h][:],
                v_sb[:, e["mt"], 2 * e["p"] + h, :],
                e["pt"][:, 512 * h:512 * (h + 1)],
                start=(e["mt"] == 0),
                stop=(e["mt"] == NT - 1),
            )
        if e["mt"] == NT - 1:
            emit_normalize(e["ot"], e["p"], e["c"], engine=last_norm_engine)

    def emit_att_chunk(p, c, filler=None):
        """Attention for head pair p, query chunk c (cols 512c..512c+512).

        The two heads of a pair sit in PE rows 0-63 / 64-127 (tile_position
        row packing); their St outputs land in the two adjacent PSUM banks
        of one [128, 1024] tile so a single wide ScalarE activation
        exponentiates both.
        """
        ot = [
            psum.tile([DH + 1, 512], F32, tag="ot", bufs=2, name=f"ot{h}")
            for h in range(2)
        ]
        for mt in range(NT):
            stp = psum.tile([128, 1024], F32, tag="stp", bufs=2, name="stp")
            for h in range(2):
                nc.tensor.matmul(
                    stp[:, 512 * h:512 * (h + 1)],
                    kt_sb[64 * h:64 * (h + 1), p, 128 * mt:128 * (mt + 1)],
                    qt_sb[64 * h:64 * (h + 1), p, 512 * c:512 * (c + 1)],
                    start=True,
                    stop=True,
                    tile_position=(64 * h, 0),
                )
            pt = work.tile([128, 1024], BF16, tag="pt", bufs=8, name="pt")
            nc.scalar.activation(
                pt[:], stp[:], mybir.ActivationFunctionType.Exp, scale=SCALE
            )
            pending.append({"ot": ot, "p": p, "c": c, "mt": mt, "pt": pt})
            if len(pending) == 4:
                flush_one()
            if filler is not None:
                filler(c, mt)

    # ---- Emission schedule ----
    # Phase B: dummy matmuls cover the initial DMA window (wqkv + xT chunk
    # 0 land ~8us in), then Kt pair 0 per xT-chunk arrival (the critical
    # path to attention start), Qt(p0, c0), and the first two V tiles.
    # Dummies bridge the xT chunk arrival gaps (the PE would otherwise
    # idle on the DMA semaphore and trip the HAM clock throttle).
    # Attention starts right after Kt(p0) chunks 0-1; chunks 2-3 stream in
    # as chunk-0 fillers, arrival-matched to their St consumers.
    phase_b = [
        ("dummy", 44),
        ("qk", "k", 0, 0), ("qk", "q", 0, 0), ("v", 0), ("v", 1),
        ("qk", "k", 0, 1),
    ]
    for u in phase_b:
        emit_unit(u)

    # Fillers (slot = mt index, NT = after last exp).  Fillers sit at the
    # START of each chunk: the first AV of a chunk waits ~1.2us for the
    # previous chunk's ot evacuation, and St(mt=1) waits for the previous
    # chunk's last exp to free its stp slot — front-loaded fillers absorb
    # both so the PE never idles (an idle PE trips the HAM clock throttle).
    # NOTE: inside emit_att_chunk the AV flush for key tiles (mt-2, mt-1)
    # is emitted BEFORE the slot-mt filler, so V tile j must sit at slot
    # <= j+1 (one earlier to cover the DVE evac latency).
    att0_fill = {
        0: {0: [("v", 2), ("v", 3)], 1: [("v", 4), ("v", 5)],
            2: [("v", 6), ("v", 7)], 3: [("qk", "k", 0, 2)],
            5: [("v", 8), ("v", 9)], 6: [("v", 10)],
            7: [("qk", "k", 0, 3)], 9: [("v", 11), ("v", 12)],
            11: [("v", 13), ("v", 14)], 13: [("v", 15), ("qk", "q", 0, 1)]},
        1: {0: [("qk", "k", 1, 0)], 1: [("qk", "q", 0, 2)],
            8: [("qk", "k", 1, 1)]},
        2: {0: [("qk", "k", 1, 2)], 1: [("qk", "q", 0, 3)],
            8: [("qk", "q", 1, 0)]},
        3: {0: [("qk", "k", 1, 3)], 8: [("qk", "q", 1, 1)]},
    }

    def att0_filler(c, mt):
        for u in att0_fill[c].get(mt, ()):
            emit_unit(u)

    for c in range(CH):
        emit_att_chunk(0, c, filler=att0_filler)

    # Phase D: attention pair 1.  Chunk 0 finishes the last Qt unit;
    # chunks 1-3 carry the out-projection for the query rows of chunk c-1
    # (complete for both pairs by then), front-loaded for the same reason.
    att1_fill = {
        0: {0: [("qk", "q", 1, 2)], 8: [("qk", "q", 1, 3)]},
    }
    # proj fillers start at slot 3: the previous chunk's normalize is only
    # emitted during this chunk's slot-2 flush, and proj reads its o rows.
    # Slots 0-2 already carry the previous chunk's final AVs + normalize,
    # so proj spreads over the middle to keep the exp stream dense.
    for c in range(1, CH):
        units = [("proj", nt, jc)
                 for nt in range(4 * (c - 1), 4 * c) for jc in range(2)]
        slots = [3, 5, 7, 9, 10, 11, 12, 13]
        att1_fill[c] = {}
        for s, u in zip(slots, units):
            att1_fill[c].setdefault(s, []).append(u)

    def att1_filler(c, mt):
        for u in att1_fill.get(c, {}).get(mt, ()):
            emit_unit(u)

    for c in range(CH):
        emit_att_chunk(1, c, filler=att1_filler)
    # Drain the carried AVs of the last chunk; its normalize copies go to
    # ScalarE (idle after the last exp) so the DVE backlog of proj
    # evacuations never delays the final proj units.
    while pending:
        flush_one(last_norm_engine="scalar")

    # Drain: last four row tiles.  The attention PSUM banks (stp/ot) are
    # free now, so the proj units rotate over three tags (6 slots) and the
    # evacuations alternate between ScalarE and DVE — both pipeline instead
    # of serializing on two qk slots.  A few dummies bridge the last
    # normalize's latency so the PE never idles into a HAM throttle.
    emit_dummies(6)
    tags = ["qk", "stp", "ot"]
    i = 0
    for nt in range(12, 16):
        for jc in range(2):
            emit_proj_unit(
                nt, jc, evac="scalar" if i % 2 else None, psum_tag=tags[i % 3]
            )
            i += 1

    ctx.close()


def _build():
    global _CACHED_NC
    if _CACHED_NC is not None:
        return _CACHED_NC
    nc = bacc.Bacc(
        "TRN2",
        target_bir_lowering=False,
        debug=False,
        enable_asserts=True,
        num_devices=N_CORES,
    )
    xt_d = nc.dram_tensor("xt", [DIM, N], BF16, kind="ExternalInput").ap()
    wqkv_d = nc.dram_tensor("wqkv", [DIM, 768], BF16, kind="ExternalInput").ap()
    wo_d = nc.dram_tensor("wo", [256, DIM], BF16, kind="ExternalInput").ap()
    out_d = nc.dram_tensor("out", [N, DIM], BF16, kind="ExternalOutput").ap()

    with tile.TileContext(nc) as tc:
        _emit_kernel(tc, xt_d, wqkv_d, wo_d, out_d)
    nc.compile()
    _CACHED_NC = nc
    return nc


def _in_maps(x, w_qkv, w_out):
    import ml_dtypes

    bf = ml_dtypes.bfloat16
    maps = []
    for c in range(N_CORES):
        b, g = divmod(c, 4)
        cols = slice(256 * g, 256 * (g + 1))
        wqkv_c = np.ascontiguousarray(
            np.concatenate(
                [
                    w_qkv[:, cols],
                    w_qkv[:, INNER:][:, cols],
                    w_qkv[:, 2 * INNER:][:, cols],
                ],
                axis=1,
            ).astype(bf)
        )
        maps.append(
            {
                "xt": np.ascontiguousarray(x[b].T.astype(bf)),
                "wqkv": wqkv_c,
                "wo": np.ascontiguousarray(w_out[cols, :].astype(bf)),
            }
        )
    return maps


def _run(x, w_qkv, w_out, b_out, trace=False):
    nc = _build()
    res = run_bass_kernel_spmd(
        nc, _in_maps(x, w_qkv, w_out), list(range(N_CORES)), trace=trace
    )
    partials = np.stack(
        [np.asarray(res.results[c]["out"], dtype=np.float32)
         for c in range(N_CORES)]
    )
    out = np.empty((B, N, DIM), dtype=np.float32)
    for b in range(B):
        out[b] = partials[4 * b:4 * b + 4].sum(axis=0) + b_out
    return out, res


def kernel(x, w_qkv, w_out, b_out):
    out, _ = _run(
        np.asarray(x, dtype=np.float32),
        np.asarray(w_qkv, dtype=np.float32),
        np.asarray(w_out, dtype=np.float32),
        np.asarray(b_out, dtype=np.float32),
    )
    return out
